# revision 2
# baseline (speedup 1.0000x reference)
"""LocallyConnected2D (B=16, 32x32, CIN=COUT=64, 3x3, pad=1) on 8 TRN2 NeuronCores.

Shard the 32 output rows across 8 cores (4 rows each); weights are repacked on
the host into a per-core, DMA-friendly layout (128 partitions, fully
contiguous). Per output pixel: 5 PSUM-accumulating matmuls — 4 with K=128
(consecutive tap pairs 2p,2p+1 stacked on partition halves; the relative
column shift between the two taps is baked into the x tile placement) plus
one K=64 matmul for tap 8. M=16 (batch), N=64 (cout); 4 pixels run
concurrently in the PE array via column tile_position. Bias added on host.

out[b,i,j,o] = sum_{c,k} x_pad[b, i+di, j+dj, c] * W[o,c,i,j,k], k=3*di+dj.

Host layouts (per core c, local row r, i = 4c+r):
  w_pairs [4, 128, 8192]: [r, 64m+cin, o*128 + j*4 + p] = W[o, cin, i, j, 2p+m]
  w_sing  [4,  64, 2048]: [r, cin,     o*32  + j      ] = W[o, cin, i, j, 8]
  xt      [6,  64,  512]: [rin, cin, j*16+b] = x_pad[b, 4c+rin, j, cin]
  out     [4, 4, 16, 8, 64]: [r, jj, b, g, o] = out[b, i, 4g+jj, o]

x tiles per row (576 = 36 cols * 16 batch; base: col j' stored at (j'+2)*16):
  pair p taps (2p, 2p+1): dj = (p%3, (p+1)%3)-ish; concretely
    P0 taps(0,1) rows(r,r)     dj(0,1)  P1 taps(2,3) rows(r,r+1) dj(2,0)
    P2 taps(4,5) rows(r+1,r+1) dj(1,2)  P3 taps(6,7) rows(r+2,r+2) dj(0,1)
  half m=1 placement shift = (dj0 - dj1) cols; lhsT AP offset = (j+dj0+1)*16.
  Tap 8 (row r+2, dj=2) reads P3's tile, partitions 0-63, offset (j+3)*16.
"""

import numpy as np

B, IH, IW, CIN = 16, 32, 32, 64
COUT, OH, OW = 64, 32, 32
NCORES, RPC = 8, 4

# per pair: (row0, row1, dj0, dj1)
PAIRS = [(0, 0, 0, 1), (0, 1, 2, 0), (1, 1, 1, 2), (2, 2, 0, 1)]

_NC = None


def _build_nc(n_reps=1):
    import concourse.bacc as bacc
    import concourse.mybir as mybir
    import concourse.tile as tile

    f32 = mybir.dt.float32
    nc = bacc.Bacc("TRN2", target_bir_lowering=False, debug=False)
    wp = nc.dram_tensor("w_pairs", [RPC, 128, 8192], f32, kind="ExternalInput")
    ws = nc.dram_tensor("w_sing", [RPC, 64, 2048], f32, kind="ExternalInput")
    xt = nc.dram_tensor("xt", [RPC + 2, 64, 512], f32, kind="ExternalInput")
    out = nc.dram_tensor("out", [RPC, 4, 16, 8, 64], f32, kind="ExternalOutput")
    wp_ap, ws_ap, xt_ap, out_ap = wp.ap(), ws.ap(), xt.ap(), out.ap()

    with tile.TileContext(nc) as tc:
        with (
            tc.tile_pool(name="wp", bufs=2) as wp_pool,
            tc.tile_pool(name="ws", bufs=2) as ws_pool,
            tc.tile_pool(name="xp", bufs=2) as xp_pool,
            tc.tile_pool(name="stage", bufs=2) as stage_pool,
            tc.tile_pool(name="psum", bufs=8, space="PSUM") as psum_pool,
        ):
            for r in [rr for _ in range(n_reps) for rr in range(RPC)]:
                wp_t = wp_pool.tile([128, 8192], f32, tag="wp")
                nc.sync.dma_start(wp_t[:], wp_ap[r])
                ws_t = ws_pool.tile([64, 2048], f32, tag="ws")
                nc.sync.dma_start(ws_t[:], ws_ap[r])

                xtiles = []
                for p, (r0, r1, dj0, dj1) in enumerate(PAIRS):
                    xti = xp_pool.tile([128, 576], f32, tag=f"x{p}")
                    # half 0: base placement, valid free [32:544)
                    nc.gpsimd.memset(xti[0:64, 0:32], 0.0)
                    nc.gpsimd.memset(xti[0:64, 544:576], 0.0)
                    nc.sync.dma_start(xti[0:64, 32:544], xt_ap[r + r0])
                    # half 1: shifted by (dj0-dj1) columns
                    lo = 32 + (dj0 - dj1) * 16
                    nc.gpsimd.memset(xti[64:128, 0:lo], 0.0)
                    if lo + 512 < 576:
                        nc.gpsimd.memset(xti[64:128, lo + 512 : 576], 0.0)
                    nc.sync.dma_start(xti[64:128, lo : lo + 512], xt_ap[r + r1])
                    xtiles.append(xti)

                stage = stage_pool.tile([128, 512], f32, tag="stage")
                wp_v = wp_t[:].rearrange("p (o q) -> p o q", q=128)
                ws_v = ws_t[:].rearrange("p (o q) -> p o q", q=32)

                for g in range(8):
                    ps = psum_pool.tile([128, 64], f32, tag="ps")
                    for t in range(5):
                        for jj in range(4):
                            j = 4 * g + jj
                            if t < 4:
                                d = PAIRS[t][2] + 1
                                lhsT = xtiles[t][:, (j + d) * 16 : (j + d + 1) * 16]
                                rhs = wp_v[:, :, 4 * j + t]
                            else:
                                lhsT = xtiles[3][0:64, (j + 3) * 16 : (j + 4) * 16]
                                rhs = ws_v[:, :, j]
                            nc.tensor.matmul(
                                ps[32 * jj : 32 * jj + 16, :],
                                lhsT,
                                rhs,
                                start=(t == 0),
                                stop=(t == 4),
                                tile_position=(0, 32 * jj),
                                skip_group_check=True,
                            )
                    for jj in range(4):
                        nc.vector.tensor_copy(
                            stage[32 * jj : 32 * jj + 16, g * 64 : (g + 1) * 64],
                            ps[32 * jj : 32 * jj + 16, :],
                        )
                for jj in range(4):
                    src = stage[32 * jj : 32 * jj + 16, :].rearrange(
                        "p (g o) -> p g o", o=64
                    )
                    nc.sync.dma_start(out_ap[r][jj], src)
    nc.compile()
    return nc


def _repack_inputs(x, weight):
    x = np.ascontiguousarray(np.asarray(x, dtype=np.float32))
    weight = np.ascontiguousarray(np.asarray(weight, dtype=np.float32))

    wt = np.ascontiguousarray(weight.transpose(2, 1, 0, 3, 4))  # [i, c, o, j, k]
    a = wt[..., :8].reshape(OH, CIN, COUT, OW, 4, 2)  # [i,c,o,j,p,m]
    wp = np.ascontiguousarray(a.transpose(0, 5, 1, 2, 3, 4)).reshape(OH, 128, 8192)
    ws = np.ascontiguousarray(wt[..., 8]).reshape(OH, CIN, 2048)

    xpad = np.zeros((IH + 2, CIN, IW, B), dtype=np.float32)
    xpad[1:33] = x.transpose(1, 3, 2, 0)  # [ih, c, j, b]

    in_maps = []
    for c in range(NCORES):
        in_maps.append(
            {
                "w_pairs": np.ascontiguousarray(wp[c * RPC : (c + 1) * RPC]),
                "w_sing": np.ascontiguousarray(ws[c * RPC : (c + 1) * RPC]),
                "xt": np.ascontiguousarray(
                    xpad[c * RPC : c * RPC + RPC + 2].reshape(RPC + 2, CIN, 512)
                ),
            }
        )
    return in_maps


def _get_nc():
    global _NC
    if _NC is None:
        _NC = _build_nc()
    return _NC


def run_spmd(in_maps, **kwargs):
    from concourse.bass_utils import run_bass_kernel_spmd

    return run_bass_kernel_spmd(
        _get_nc(), in_maps, core_ids=list(range(NCORES)), **kwargs
    )


def kernel(x, weight, bias, _results=None):
    if _results is None:
        _results = run_spmd(_repack_inputs(x, weight)).results
    arr = np.stack([r["out"] for r in _results])  # [core, r, jj, b, g, o]
    out = arr.transpose(3, 0, 1, 4, 2, 5).reshape(B, OH, OW, COUT)
    return out + np.asarray(bias, dtype=np.float32)[None]



# revision 10
# speedup vs baseline: 3.3922x; 3.3922x over previous
"""LocallyConnected2D (B=16, 32x32, CIN=COUT=64, 3x3, pad=1) on 8 TRN2 NeuronCores.

Shard the 32 output rows across 8 cores (4 rows each); all tensors fp16 on
device (fp32 PSUM accumulate), fp32 finish on host.

Tap pairing (k = 3*di + dj): pairs (0,3), (1,4), (2,5) put taps with the SAME
column shift dj=t on the two partition halves (rows r and r+1), so the lhsT
for a K=128 matmul is just two consecutive padded x rows stacked — one
contiguous DMA, no shifted copies. Row r+2's taps 6,7,8 run as three K=64
solo matmuls. Per pixel: 6 PSUM-accumulating matmuls, M=16 (batch), N=64
(cout); 4 pixels run concurrently in the PE array via column tile_position.

out[b,i,j,o] = sum_{c,k} x_pad[b, i+di, j+dj, c] * W[o,c,i,j,k].

Host layouts (per core c, local row r, i = 4c+r, j = 4g+jj, m = half):
  w_pairs [4, 128, 6144]: [64m+cin, j*192 + o*3 + t] = W[o, cin, i, j, t+3m]
  w_solo  [4, 128, 3072]: [64(j%2)+cin, (j//2)*192 + o*3 + t] = W[o, cin, i, j, 6+t]
  xt      [384, 512]:     [rin*64+cin, j*16+b] = x_pad[b, 4c+rin, j, cin]
  out     [4, 16, 2048] fp16: [jj, b, r*512 + g*64 + o] = out[b, i, j, o]

x panels V(k) [128, 544], k=0..4: partitions = x rows (k, k+1), padded col c
stored at (c+1)*16 (memset zero borders for c=-1, 32); S5 [64,544] = row 5.
Pair t lhsT = V(r)[:, (j+t)*16:+16]. Solo t: row r+2 = V(r+2)[0:64] (j even,
r<3), V(r+1)[64:128] (j odd), S5[0:64] (j even, r=3).

PSUM: one [128, 512] bank per r accumulates all 8 column groups; 4 DVE
casts f32->fp16 per r into stage [128, 2048]; 4 output DMAs per core.
Weight DMAs ride the SP HWDGE ring; x/out DMAs ride the ACT ring.
"""

import numpy as np

B, IH, IW, CIN = 16, 32, 32, 64
COUT, OH, OW = 64, 32, 32
NCORES, RPC = 8, 4

_NC = None


def _build_nc(n_reps=1):
    import concourse.bacc as bacc
    import concourse.mybir as mybir
    import concourse.tile as tile

    f16 = mybir.dt.float16
    f32 = mybir.dt.float32
    nc = bacc.Bacc("TRN2", target_bir_lowering=False, debug=False)
    wp = nc.dram_tensor("w_pairs", [RPC, 128, 6144], f16, kind="ExternalInput")
    wso = nc.dram_tensor("w_solo", [RPC, 128, 3072], f16, kind="ExternalInput")
    xt = nc.dram_tensor("xt", [384, 512], f16, kind="ExternalInput")
    out = nc.dram_tensor("out", [4, 16, RPC * 512], f16, kind="ExternalOutput")
    wp_ap, wso_ap, xt_ap, out_ap = wp.ap(), wso.ap(), xt.ap(), out.ap()

    with tile.TileContext(nc) as tc:
        with (
            tc.tile_pool(name="wp", bufs=2) as wp_pool,
            tc.tile_pool(name="wso", bufs=2) as wso_pool,
            tc.tile_pool(name="vx", bufs=1) as vx_pool,
            tc.tile_pool(name="stage", bufs=1) as stage_pool,
            tc.tile_pool(name="psum", bufs=4, space="PSUM") as psum_pool,
        ):
            for rep in range(n_reps):
                # x panels: V(k) = padded x rows (k, k+1), col c at (c+1)*16
                vs = []
                for k in range(5):
                    v = vx_pool.tile([128, 544], f16, tag=f"v{k}")
                    nc.gpsimd.memset(v[:, 0:16], 0.0)
                    nc.gpsimd.memset(v[:, 528:544], 0.0)
                    nc.scalar.dma_start(v[:, 16:528], xt_ap[64 * k : 64 * k + 128])
                    vs.append(v)
                s5 = vx_pool.tile([64, 544], f16, tag="s5")
                nc.gpsimd.memset(s5[:, 0:16], 0.0)
                nc.gpsimd.memset(s5[:, 528:544], 0.0)
                nc.scalar.dma_start(s5[:, 16:528], xt_ap[320:384])

                stage = stage_pool.tile([128, 2048], f16, tag="stage")
                for r in range(RPC):
                    wp_t = wp_pool.tile([128, 6144], f16, tag="wp")
                    wso_t = wso_pool.tile([128, 3072], f16, tag="wso")
                    # split so the first column groups can start early;
                    # pair/solo weights ride different HWDGE rings
                    nc.sync.dma_start(wp_t[:, 0:3072], wp_ap[r][:, 0:3072])
                    nc.scalar.dma_start(wso_t[:, 0:1536], wso_ap[r][:, 0:1536])
                    nc.sync.dma_start(wp_t[:, 3072:6144], wp_ap[r][:, 3072:6144])
                    nc.scalar.dma_start(wso_t[:, 1536:3072], wso_ap[r][:, 1536:3072])

                    wp_v = wp_t[:].rearrange("p (j o t) -> p j o t", o=64, t=3)
                    wso_v = wso_t[:].rearrange("p (h o t) -> p h o t", o=64, t=3)

                    ps = psum_pool.tile([128, 512], f32, tag="ps")
                    for g in range(8):
                        for t in range(6):
                            for jj in range(4):
                                j = 4 * g + jj
                                if t < 3:
                                    lhsT = vs[r][:, (j + t) * 16 : (j + t + 1) * 16]
                                    rhs = wp_v[:, j, :, t]
                                    tp = (0, 32 * jj)
                                else:
                                    dj = t - 3
                                    off = (j + dj) * 16
                                    if j % 2 == 1:
                                        lhsT = vs[r + 1][64:128, off : off + 16]
                                        rhs = wso_v[64:128, j // 2, :, dj]
                                        tp = (64, 32 * jj)
                                    else:
                                        src = s5 if r == 3 else vs[r + 2]
                                        lhsT = src[0:64, off : off + 16]
                                        rhs = wso_v[0:64, j // 2, :, dj]
                                        tp = (0, 32 * jj)
                                nc.tensor.matmul(
                                    ps[32 * jj : 32 * jj + 16, 64 * g : 64 * g + 64],
                                    lhsT,
                                    rhs,
                                    start=(t == 0),
                                    stop=(t == 5),
                                    tile_position=tp,
                                    skip_group_check=True,
                                )
                    for jj in range(4):
                        nc.vector.tensor_copy(
                            stage[32 * jj : 32 * jj + 16, r * 512 : (r + 1) * 512],
                            ps[32 * jj : 32 * jj + 16, :],
                        )
                        nc.scalar.dma_start(
                            out_ap[jj][:, r * 512 : (r + 1) * 512],
                            stage[32 * jj : 32 * jj + 16, r * 512 : (r + 1) * 512],
                        )
    nc.compile()
    return nc


def _repack_inputs(x, weight):
    x = np.asarray(x, dtype=np.float32)
    weight = np.asarray(weight, dtype=np.float32)

    wt = np.ascontiguousarray(weight.transpose(2, 1, 0, 3, 4))  # [i, c, o, j, k]
    a = wt[..., :6].reshape(OH, CIN, COUT, OW, 2, 3)  # [i,c,o,j,m,t]
    wpair = (
        np.ascontiguousarray(a.transpose(0, 4, 1, 3, 2, 5))  # [i,m,c,j,o,t]
        .reshape(OH, 128, 6144)
        .astype(np.float16)
    )
    b6 = wt[..., 6:9].reshape(OH, CIN, COUT, 16, 2, 3)  # [i,c,o,jh,jp,t]
    wsolo = (
        np.ascontiguousarray(b6.transpose(0, 4, 1, 3, 2, 5))  # [i,jp,c,jh,o,t]
        .reshape(OH, 128, 3072)
        .astype(np.float16)
    )

    xpad = np.zeros((IH + 2, CIN, IW, B), dtype=np.float16)
    xpad[1:33] = x.transpose(1, 3, 2, 0)  # [ih, c, j, b]

    in_maps = []
    for c in range(NCORES):
        in_maps.append(
            {
                "w_pairs": np.ascontiguousarray(wpair[c * RPC : (c + 1) * RPC]),
                "w_solo": np.ascontiguousarray(wsolo[c * RPC : (c + 1) * RPC]),
                "xt": np.ascontiguousarray(
                    xpad[c * RPC : c * RPC + RPC + 2].reshape(384, 512)
                ),
            }
        )
    return in_maps


def _get_nc():
    global _NC
    if _NC is None:
        _NC = _build_nc()
    return _NC


def run_spmd(in_maps, **kwargs):
    from concourse.bass_utils import run_bass_kernel_spmd

    return run_bass_kernel_spmd(
        _get_nc(), in_maps, core_ids=list(range(NCORES)), **kwargs
    )


def kernel(x, weight, bias, _results=None):
    if _results is None:
        _results = run_spmd(_repack_inputs(x, weight)).results
    arr = np.stack([r["out"] for r in _results]).astype(np.float32)
    arr = arr.reshape(NCORES, 4, 16, RPC, 8, 64)
    # arr: [core, jj, b, r, g, o] -> out[b, 4c+r, 4g+jj, o]
    out = arr.transpose(2, 0, 3, 4, 1, 5).reshape(B, OH, OW, COUT)
    return out + np.asarray(bias, dtype=np.float32)[None]


# revision 11
# speedup vs baseline: 3.6626x; 1.0797x over previous
"""LocallyConnected2D (B=16, 32x32, CIN=COUT=64, 3x3, pad=1) on 8 TRN2 NeuronCores.

Shard the 32 output rows across 8 cores (4 rows each); all tensors fp16 on
device (fp32 PSUM accumulate), fp32 finish on host.

Tap pairing (k = 3*di + dj): pairs (0,3), (1,4), (2,5) put taps with the SAME
column shift dj=t on the two partition halves (rows r and r+1), so the lhsT
for a K=128 matmul is just two consecutive padded x rows stacked — one
contiguous DMA, no shifted copies. Row r+2's taps 6,7,8 run as three K=64
solo matmuls. Per pixel: 6 PSUM-accumulating matmuls, M=16 (batch), N=64
(cout); 4 pixels run concurrently in the PE array via column tile_position.

out[b,i,j,o] = sum_{c,k} x_pad[b, i+di, j+dj, c] * W[o,c,i,j,k].

Host layouts (per core c, local row r, i = 4c+r, j = 4g+jj, m = half):
  w_pairs [4, 128, 6144]: [64m+cin, j*192 + t*64 + o] = W[o, cin, i, j, t+3m]
  w_solo  [4, 128, 3072]: [64(j%2)+cin, (j//2)*192 + t*64 + o] = W[o, cin, i, j, 6+t]
  xt      [384, 512]:     [rin*64+cin, j*16+b] = x_pad[b, 4c+rin, j, cin]
  out     [4, 16, 2048] fp16: [jj, b, r*512 + g*64 + o] = out[b, i, j, o]

x panels V(k) [128, 544], k=0..4: partitions = x rows (k, k+1), padded col c
stored at (c+1)*16 (memset zero borders for c=-1, 32); S5 [64,544] = row 5.
Pair t lhsT = V(r)[:, (j+t)*16:+16]. Solo t: row r+2 = V(r+2)[0:64] (j even,
r<3), V(r+1)[64:128] (j odd), S5[0:64] (j even, r=3).

PSUM: one [128, 512] bank per r accumulates all 8 column groups; 4 DVE
casts f32->fp16 per r into stage [128, 2048]; 4 output DMAs per core.
Weight DMAs ride the SP HWDGE ring; x/out DMAs ride the ACT ring.
"""

import numpy as np

B, IH, IW, CIN = 16, 32, 32, 64
COUT, OH, OW = 64, 32, 32
NCORES, RPC = 8, 4

_NC = None


def _build_nc(n_reps=1):
    import concourse.bacc as bacc
    import concourse.mybir as mybir
    import concourse.tile as tile

    f16 = mybir.dt.float16
    f32 = mybir.dt.float32
    nc = bacc.Bacc("TRN2", target_bir_lowering=False, debug=False)
    wp = nc.dram_tensor("w_pairs", [RPC, 128, 6144], f16, kind="ExternalInput")
    wso = nc.dram_tensor("w_solo", [RPC, 128, 3072], f16, kind="ExternalInput")
    xt = nc.dram_tensor("xt", [384, 512], f16, kind="ExternalInput")
    out = nc.dram_tensor("out", [4, 16, RPC * 512], f16, kind="ExternalOutput")
    wp_ap, wso_ap, xt_ap, out_ap = wp.ap(), wso.ap(), xt.ap(), out.ap()

    with tile.TileContext(nc) as tc:
        with (
            tc.tile_pool(name="wp", bufs=3) as wp_pool,
            tc.tile_pool(name="wso", bufs=3) as wso_pool,
            tc.tile_pool(name="vx", bufs=1) as vx_pool,
            tc.tile_pool(name="stage", bufs=1) as stage_pool,
            tc.tile_pool(name="psum", bufs=4, space="PSUM") as psum_pool,
        ):
            for rep in range(n_reps):
                # x panels: V(k) = padded x rows (k, k+1), col c at (c+1)*16
                vs = []
                for k in range(5):
                    v = vx_pool.tile([128, 544], f16, tag=f"v{k}")
                    nc.gpsimd.memset(v[:, 0:16], 0.0)
                    nc.gpsimd.memset(v[:, 528:544], 0.0)
                    nc.scalar.dma_start(v[:, 16:528], xt_ap[64 * k : 64 * k + 128])
                    vs.append(v)
                s5 = vx_pool.tile([64, 544], f16, tag="s5")
                nc.gpsimd.memset(s5[:, 0:16], 0.0)
                nc.gpsimd.memset(s5[:, 528:544], 0.0)
                nc.scalar.dma_start(s5[:, 16:528], xt_ap[320:384])

                stage = stage_pool.tile([128, 2048], f16, tag="stage")
                for r in range(RPC):
                    wp_t = wp_pool.tile([128, 6144], f16, tag="wp")
                    wso_t = wso_pool.tile([128, 3072], f16, tag="wso")
                    # split so the first column groups can start early;
                    # pair/solo weights ride different HWDGE rings
                    nc.sync.dma_start(wp_t[:, 0:3072], wp_ap[r][:, 0:3072])
                    nc.scalar.dma_start(wso_t[:, 0:1536], wso_ap[r][:, 0:1536])
                    nc.sync.dma_start(wp_t[:, 3072:6144], wp_ap[r][:, 3072:6144])
                    nc.scalar.dma_start(wso_t[:, 1536:3072], wso_ap[r][:, 1536:3072])

                    wp_v = wp_t[:].rearrange("p (j t o) -> p j t o", o=64, t=3)
                    wso_v = wso_t[:].rearrange("p (h t o) -> p h t o", o=64, t=3)

                    ps = psum_pool.tile([128, 512], f32, tag="ps")
                    for g in range(8):
                        for t in range(6):
                            for jj in range(4):
                                j = 4 * g + jj
                                if t < 3:
                                    lhsT = vs[r][:, (j + t) * 16 : (j + t + 1) * 16]
                                    rhs = wp_v[:, j, t, :]
                                    tp = (0, 32 * jj)
                                else:
                                    dj = t - 3
                                    off = (j + dj) * 16
                                    if j % 2 == 1:
                                        lhsT = vs[r + 1][64:128, off : off + 16]
                                        rhs = wso_v[64:128, j // 2, dj, :]
                                        tp = (64, 32 * jj)
                                    else:
                                        src = s5 if r == 3 else vs[r + 2]
                                        lhsT = src[0:64, off : off + 16]
                                        rhs = wso_v[0:64, j // 2, dj, :]
                                        tp = (0, 32 * jj)
                                nc.tensor.matmul(
                                    ps[32 * jj : 32 * jj + 16, 64 * g : 64 * g + 64],
                                    lhsT,
                                    rhs,
                                    start=(t == 0),
                                    stop=(t == 5),
                                    tile_position=tp,
                                    skip_group_check=True,
                                )
                    for jj in range(4):
                        nc.vector.tensor_copy(
                            stage[32 * jj : 32 * jj + 16, r * 512 : (r + 1) * 512],
                            ps[32 * jj : 32 * jj + 16, :],
                        )
                        nc.scalar.dma_start(
                            out_ap[jj][:, r * 512 : (r + 1) * 512],
                            stage[32 * jj : 32 * jj + 16, r * 512 : (r + 1) * 512],
                        )
    nc.compile()
    return nc


def _repack_inputs(x, weight):
    x = np.asarray(x, dtype=np.float32)
    weight = np.asarray(weight, dtype=np.float32)

    wt = np.ascontiguousarray(weight.transpose(2, 1, 0, 3, 4))  # [i, c, o, j, k]
    a = wt[..., :6].reshape(OH, CIN, COUT, OW, 2, 3)  # [i,c,o,j,m,t]
    wpair = (
        np.ascontiguousarray(a.transpose(0, 4, 1, 3, 5, 2))  # [i,m,c,j,t,o]
        .reshape(OH, 128, 6144)
        .astype(np.float16)
    )
    b6 = wt[..., 6:9].reshape(OH, CIN, COUT, 16, 2, 3)  # [i,c,o,jh,jp,t]
    wsolo = (
        np.ascontiguousarray(b6.transpose(0, 4, 1, 3, 5, 2))  # [i,jp,c,jh,t,o]
        .reshape(OH, 128, 3072)
        .astype(np.float16)
    )

    xpad = np.zeros((IH + 2, CIN, IW, B), dtype=np.float16)
    xpad[1:33] = x.transpose(1, 3, 2, 0)  # [ih, c, j, b]

    in_maps = []
    for c in range(NCORES):
        in_maps.append(
            {
                "w_pairs": np.ascontiguousarray(wpair[c * RPC : (c + 1) * RPC]),
                "w_solo": np.ascontiguousarray(wsolo[c * RPC : (c + 1) * RPC]),
                "xt": np.ascontiguousarray(
                    xpad[c * RPC : c * RPC + RPC + 2].reshape(384, 512)
                ),
            }
        )
    return in_maps


def _get_nc():
    global _NC
    if _NC is None:
        _NC = _build_nc()
    return _NC


def run_spmd(in_maps, **kwargs):
    from concourse.bass_utils import run_bass_kernel_spmd

    return run_bass_kernel_spmd(
        _get_nc(), in_maps, core_ids=list(range(NCORES)), **kwargs
    )


def kernel(x, weight, bias, _results=None):
    if _results is None:
        _results = run_spmd(_repack_inputs(x, weight)).results
    arr = np.stack([r["out"] for r in _results]).astype(np.float32)
    arr = arr.reshape(NCORES, 4, 16, RPC, 8, 64)
    # arr: [core, jj, b, r, g, o] -> out[b, 4c+r, 4g+jj, o]
    out = arr.transpose(2, 0, 3, 4, 1, 5).reshape(B, OH, OW, COUT)
    return out + np.asarray(bias, dtype=np.float32)[None]
